# revision 4
# baseline (speedup 1.0000x reference)
"""Trainium2 Bass kernel for nn_LoraInjectedLinear (moe_routing).

Computation (per chunk b of 16):
    idx_b  = lora_id[b] // 4, active_b = lora_id[b] >= 0
    out[b] = x[b] @ W.T + active_b * SCALE * (x[b] @ Wd[idx_b].T) @ Wu[idx_b].T

Strategy:
  - Host folds the rank-4 LoRA pair into a per-chunk fused weight:
        W_aug[b] = W + active_b * SCALE * Wu[idx_b] @ Wd[idx_b]
    and pre-transposes both the fused weight ([d, o] layout) and x
    ([d, t] layout) so the device kernel is a pure batched GEMM with the
    contraction dim on SBUF partitions (no on-device transposes).
  - Data parallel across 8 NeuronCores: 2 chunks per core.
  - All device I/O in fp16 (x, fused W, out). PE streams fp16 at the
    same 1 cycle/row as fp32r but LDWEIGHTS takes half the rows and
    HBM traffic halves; PSUM accumulates in fp32, host casts back.
  - Matmul loops ordered k-tile-outer / out-chunk-inner so 3
    consecutive matmuls share one stationary tile.
"""

import os

import numpy as np

G = 16  # chunks
T = 4096  # tokens per chunk
D_IN = 1280
D_OUT = 1280
RANK = 4
LORA_STRIDE = 4
SCALE = 1.0

N_CORES = 8
CPC = G // N_CORES  # chunks per core = 2

P = 128
D_TILES = D_IN // P  # 10
T_BLK = 512  # tokens per x DMA block
T_SUB = T_BLK // P  # 4 subtiles of 128 tokens
N_BLKS = T // T_BLK  # 8 blocks per chunk
O_CHUNKS = [(0, 512), (512, 512), (1024, 256)]  # N-slices of D_OUT

_NC = None


def _build():
    global _NC
    if _NC is not None:
        return _NC

    import concourse.mybir as mybir
    from concourse import bacc
    from concourse.tile import TileContext

    f16 = mybir.dt.float16
    f32 = mybir.dt.float32

    nc = bacc.Bacc()
    xT = nc.declare_dram_parameter("xT", [CPC, D_IN, T], f16, isOutput=False)
    wT = nc.declare_dram_parameter("wT", [CPC, D_IN, D_OUT], f16, isOutput=False)
    out = nc.declare_dram_parameter("out", [CPC, T, D_OUT], f16, isOutput=True)

    with TileContext(nc) as tc:
        with (
            tc.tile_pool(name="wpool", bufs=2 * D_TILES) as wpool,
            tc.tile_pool(name="xpool", bufs=2) as xpool,
            tc.tile_pool(name="opool", bufs=5) as opool,
            tc.tile_pool(name="pspool", bufs=8, space="PSUM") as pspool,
        ):
            # PE warm-up: ~16 matmuls on a zeroed scratch tile, issued
            # before any data arrives. They execute during the ~10 us
            # DMA bring-up window so the PE's DVFS ramp (0.65 -> 2.4
            # GHz over ~3 us of busy time) happens on throwaway work
            # and the first real matmul runs at full clock.
            warm = wpool.tile([P, 512], f16, name="warm", tag="warm")
            nc.gpsimd.memset(warm[:], 0)
            ps_warm = pspool.tile([P, 512], f32, name="ps_warm", tag="ps")
            for _ in range(16):
                nc.tensor.matmul(
                    ps_warm[:],
                    lhsT=warm[:, :P],
                    rhs=warm[:],
                    start=True,
                    stop=True,
                )

            # Weights split across the ACT and GPSIMD HWDGE rings (5
            # tiles each) so doorbell issue doesn't serialize behind one
            # engine; one SBUF tile per d-tile so each matmul depends
            # only on its own 325 KB DMA and the first block's d-outer
            # ramp can track the arriving stream. Chunk 0's weights load
            # immediately; chunk 1's are deferred to mid-chunk-0 so they
            # don't steal SDMA bandwidth from the critical first x block
            # (all queues share 16 SDMA engines).
            def load_weights(c):
                wsrc = wT.ap()[c].rearrange("(n p) o -> p n o", p=P)
                row = []
                for n in range(D_TILES):
                    wtn = wpool.tile([P, D_OUT], f16, name=f"w_{c}_{n}", tag="wt")
                    eng = nc.scalar if n % 2 == 0 else nc.gpsimd
                    eng.dma_start(wtn[:], wsrc[:, n, :])
                    row.append(wtn)
                return row

            wts = {0: load_weights(0)}

            def copyback_and_store(ps_row, c, j, s):
                ot = opool.tile([P, D_OUT], f16)
                for oi, (o0, ow) in enumerate(O_CHUNKS):
                    if oi == 1:
                        nc.vector.tensor_copy(ot[:, o0 : o0 + ow], ps_row[oi][:, :ow])
                    else:
                        nc.scalar.copy(ot[:, o0 : o0 + ow], ps_row[oi][:, :ow])
                dst = out.ap()[c, (j * T_SUB + s) * P : (j * T_SUB + s + 1) * P, :]
                last = c == CPC - 1 and j == N_BLKS - 1 and s == T_SUB - 1
                if last:
                    # final subtile: store per o-chunk so the tail DMA
                    # starts as soon as each copy lands (oi=1 finishes
                    # first on the vector engine, oi=2 last)
                    for oi in (1, 0, 2):
                        o0, ow = O_CHUNKS[oi]
                        nc.scalar.dma_start(
                            dst[:, o0 : o0 + ow], ot[:, o0 : o0 + ow]
                        )
                else:
                    nc.scalar.dma_start(dst, ot[:])

            for c in range(CPC):
                wt = wts[c]
                for j in range(N_BLKS):
                    xt = xpool.tile([P, D_TILES, T_BLK], f16)
                    xsrc = xT.ap()[c, :, j * T_BLK : (j + 1) * T_BLK].rearrange(
                        "(n p) t -> p n t", p=P
                    )
                    if c == 0 and j == 0:
                        # per-subtile slices so the ramp's first wave only
                        # waits for 650 KB, not the whole 1.3 MB block
                        for s in range(T_SUB):
                            nc.sync.dma_start(
                                xt[:, :, s * P : (s + 1) * P],
                                xsrc[:, :, s * P : (s + 1) * P],
                            )
                    else:
                        nc.sync.dma_start(xt[:], xsrc)
                    if c == 0 and j == 2:
                        wts[1] = load_weights(1)

                    def mm_subtile(s):
                        # k-tile outer, out-chunk inner: the 3 matmuls of
                        # one k-tile share the same stationary lhsT.
                        ps_row = [
                            pspool.tile([P, 512], f32, name="ps", tag="ps")
                            for _ in O_CHUNKS
                        ]
                        for n in range(D_TILES):
                            for oi, (o0, ow) in enumerate(O_CHUNKS):
                                nc.tensor.matmul(
                                    ps_row[oi][:, :ow],
                                    lhsT=xt[:, n, s * P : (s + 1) * P],
                                    rhs=wt[n][:, o0 : o0 + ow],
                                    start=(n == 0),
                                    stop=(n == D_TILES - 1),
                                )
                        return ps_row

                    if c == 0 and j == 0:
                        # Ramp: d-outer waves over 8 psum banks (subtiles
                        # 0-1 all o-chunks + subtiles 2-3 o-chunk 0) so the
                        # PE keeps pace with the streaming weight d-tiles.
                        groups = [(s, oi) for s in range(2) for oi in range(3)]
                        psr = {
                            (s, oi): pspool.tile(
                                [P, 512], f32, name=f"ps_r{s}_{oi}", tag="ps"
                            )
                            for (s, oi) in groups
                        }
                        for n in range(D_TILES):
                            for s, oi in groups:
                                ow = O_CHUNKS[oi][1]
                                nc.tensor.matmul(
                                    psr[(s, oi)][:, :ow],
                                    lhsT=xt[:, n, s * P : (s + 1) * P],
                                    rhs=wt[n][:, oi * 512 : oi * 512 + ow],
                                    start=(n == 0),
                                    stop=(n == D_TILES - 1),
                                )
                        for s in range(2):
                            copyback_and_store(
                                [psr[(s, oi)] for oi in range(3)], c, j, s
                            )
                        for s in (2, 3):
                            copyback_and_store(mm_subtile(s), c, j, s)
                    else:
                        for s in range(T_SUB):
                            copyback_and_store(mm_subtile(s), c, j, s)
    nc.finalize()
    _NC = nc
    return nc


def _host_prep(x, lora_id, W, Wd, Wu):
    x = np.asarray(x, dtype=np.float32)
    lora_id = np.asarray(lora_id)
    W = np.asarray(W, dtype=np.float32)
    Wd = np.asarray(Wd, dtype=np.float32)
    Wu = np.asarray(Wu, dtype=np.float32)

    idx = lora_id.astype(np.int64) // LORA_STRIDE
    active = lora_id >= 0
    safe_idx = np.where(active, idx, 0)

    WT = np.ascontiguousarray(W.T)  # [d, o]
    waugT = np.empty((G, D_IN, D_OUT), dtype=np.float16)
    for b in range(G):
        if active[b]:
            i = int(safe_idx[b])
            # (Wu[i] @ Wd[i]).T = Wd[i].T @ Wu[i].T : [d, o]
            waugT[b] = WT + SCALE * (Wd[i].T @ Wu[i].T)
        else:
            waugT[b] = WT

    # [G, d, t] — contraction dim first so SBUF tiles need no transpose
    xT = np.ascontiguousarray(x.transpose(0, 2, 1)).astype(np.float16)
    return xT, waugT


def kernel(x, lora_id, W, Wd, Wu):
    from concourse.bass_utils import run_bass_kernel_spmd

    xT, waugT = _host_prep(x, lora_id, W, Wd, Wu)

    nc = _build()
    in_maps = [
        {"xT": xT[k * CPC : (k + 1) * CPC], "wT": waugT[k * CPC : (k + 1) * CPC]}
        for k in range(N_CORES)
    ]
    trace = bool(os.environ.get("KERNEL_PROFILE"))
    kwargs = {}
    if trace and os.environ.get("KERNEL_PROFILE_DIR"):
        kwargs["tmpdir"] = os.environ["KERNEL_PROFILE_DIR"]
    res = run_bass_kernel_spmd(nc, in_maps, list(range(N_CORES)), trace=trace, **kwargs)
    if trace:
        kernel.last_results = res
        print(f"HW exec time: {res.exec_time_ns} ns")
    return np.concatenate(
        [res.results[k]["out"] for k in range(N_CORES)], axis=0
    ).astype(np.float32)


# revision 6
# speedup vs baseline: 1.2300x; 1.2300x over previous
"""Trainium2 Bass kernel for nn_LoraInjectedLinear (moe_routing).

Computation (per chunk b of 16):
    idx_b  = lora_id[b] // 4, active_b = lora_id[b] >= 0
    out[b] = x[b] @ W.T + active_b * SCALE * (x[b] @ Wd[idx_b].T) @ Wu[idx_b].T

Strategy:
  - Host folds the rank-4 LoRA pair into a per-chunk fused weight:
        W_aug[b] = W + active_b * SCALE * Wu[idx_b] @ Wd[idx_b]
    and pre-transposes both the fused weight ([d, o] layout) and x
    ([d, t] layout) so the device kernel is a pure batched GEMM with the
    contraction dim on SBUF partitions (no on-device transposes).
  - Data parallel across 8 NeuronCores: 2 chunks per core.
  - All device I/O in fp16 (x, fused W, out). PE streams fp16 at the
    same 1 cycle/row as fp32r but LDWEIGHTS takes half the rows and
    HBM traffic halves; PSUM accumulates in fp32, host casts back.
  - Matmul loops ordered k-tile-outer / out-chunk-inner so 3
    consecutive matmuls share one stationary tile.
"""

import os

import numpy as np

G = 16  # chunks
T = 4096  # tokens per chunk
D_IN = 1280
D_OUT = 1280
RANK = 4
LORA_STRIDE = 4
SCALE = 1.0

N_CORES = 8
CPC = G // N_CORES  # chunks per core = 2

P = 128
D_TILES = D_IN // P  # 10
T_BLK = 512  # tokens per x DMA block
T_SUB = T_BLK // P  # 4 subtiles of 128 tokens
N_BLKS = T // T_BLK  # 8 blocks per chunk
O_CHUNKS = [(0, 512), (512, 512), (1024, 256)]  # N-slices of D_OUT

_NC = None


def _build():
    global _NC
    if _NC is not None:
        return _NC

    import concourse.mybir as mybir
    from concourse import bacc
    from concourse.tile import TileContext

    f16 = mybir.dt.float16
    f32 = mybir.dt.float32

    nc = bacc.Bacc()
    xT = nc.declare_dram_parameter("xT", [CPC, D_IN, T], f16, isOutput=False)
    wT = nc.declare_dram_parameter("wT", [CPC, D_IN, D_OUT], f16, isOutput=False)
    out = nc.declare_dram_parameter("out", [CPC, T, D_OUT], f16, isOutput=True)

    with TileContext(nc) as tc:
        with (
            tc.tile_pool(name="wpool", bufs=2 * D_TILES) as wpool,
            tc.tile_pool(name="xpool", bufs=2) as xpool,
            tc.tile_pool(name="opool", bufs=5) as opool,
            tc.tile_pool(name="pspool", bufs=8, space="PSUM") as pspool,
        ):
            # Small PE warm-up on a zeroed scratch tile: if the PE boots
            # before the first data DMAs land, these spin the DVFS ramp
            # on throwaway work; if data is already there they cost ~1us.
            warm = wpool.tile([P, P], f16, name="warm", tag="warm")
            nc.vector.memset(warm[:], 0)
            ps_warm = pspool.tile([P, 512], f32, name="ps_warm", tag="ps")
            for _ in range(6):
                nc.tensor.matmul(
                    ps_warm[:, :P],
                    lhsT=warm[:],
                    rhs=warm[:],
                    start=True,
                    stop=True,
                )

            # First x block, sliced per d-tile on the SYNC ring: 1 KB
            # descriptor runs (>=512 B keeps full DMA rate) and the
            # ramp's wave n depends only on slice n, so compute tracks
            # the arriving stream at d-tile granularity.
            xt0 = xpool.tile([P, D_TILES, T_BLK], f16)
            xsrc0 = xT.ap()[0, :, 0:T_BLK].rearrange("(n p) t -> p n t", p=P)
            for n in range(D_TILES):
                nc.sync.dma_start(xt0[:, n, :], xsrc0[:, n, :])

            # Weights split across the ACT and SYNC HWDGE rings (both
            # hardware-DGE; the GpSimd ring is software-DGE and far too
            # slow) so doorbell issue doesn't serialize behind one
            # engine; one SBUF tile per d-tile so each matmul depends
            # only on its own 325 KB DMA. Chunk 0's weights load
            # immediately; chunk 1's are deferred to mid-chunk-0 so they
            # don't steal SDMA bandwidth from the critical first x block
            # (all queues share 16 SDMA engines).
            def load_weights(c):
                wsrc = wT.ap()[c].rearrange("(n p) o -> p n o", p=P)
                row = []
                for n in range(D_TILES):
                    wtn = wpool.tile([P, D_OUT], f16, name=f"w_{c}_{n}", tag="wt")
                    eng = nc.scalar if n < 5 else nc.sync
                    eng.dma_start(wtn[:], wsrc[:, n, :])
                    row.append(wtn)
                return row

            wts = {0: load_weights(0)}

            def copyback_and_store(ps_row, c, j, s):
                ot = opool.tile([P, D_OUT], f16)
                for oi, (o0, ow) in enumerate(O_CHUNKS):
                    if oi == 1:
                        nc.vector.tensor_copy(ot[:, o0 : o0 + ow], ps_row[oi][:, :ow])
                    else:
                        nc.scalar.copy(ot[:, o0 : o0 + ow], ps_row[oi][:, :ow])
                dst = out.ap()[c, (j * T_SUB + s) * P : (j * T_SUB + s + 1) * P, :]
                last = c == CPC - 1 and j == N_BLKS - 1 and s == T_SUB - 1
                if last:
                    # final subtile: store per o-chunk so the tail DMA
                    # starts as soon as each copy lands (oi=1 finishes
                    # first on the vector engine, oi=2 last)
                    for oi in (1, 0, 2):
                        o0, ow = O_CHUNKS[oi]
                        nc.scalar.dma_start(
                            dst[:, o0 : o0 + ow], ot[:, o0 : o0 + ow]
                        )
                else:
                    nc.scalar.dma_start(dst, ot[:])

            for c in range(CPC):
                wt = wts[c]
                for j in range(N_BLKS):
                    if c == 0 and j == 0:
                        xt = xt0
                    else:
                        xt = xpool.tile([P, D_TILES, T_BLK], f16)
                        xsrc = xT.ap()[c, :, j * T_BLK : (j + 1) * T_BLK].rearrange(
                            "(n p) t -> p n t", p=P
                        )
                        nc.sync.dma_start(xt[:], xsrc)
                    if c == 0 and j == 2:
                        wts[1] = load_weights(1)

                    def mm_subtile(s):
                        # k-tile outer, out-chunk inner: the 3 matmuls of
                        # one k-tile share the same stationary lhsT.
                        ps_row = [
                            pspool.tile([P, 512], f32, name="ps", tag="ps")
                            for _ in O_CHUNKS
                        ]
                        for n in range(D_TILES):
                            for oi, (o0, ow) in enumerate(O_CHUNKS):
                                nc.tensor.matmul(
                                    ps_row[oi][:, :ow],
                                    lhsT=xt[:, n, s * P : (s + 1) * P],
                                    rhs=wt[n][:, o0 : o0 + ow],
                                    start=(n == 0),
                                    stop=(n == D_TILES - 1),
                                )
                        return ps_row

                    if c == 0 and j == 0:
                        # Ramp: d-outer waves over 8 psum banks (subtiles
                        # 0-1 all o-chunks + subtiles 2-3 o-chunk 0) so the
                        # PE keeps pace with the streaming weight d-tiles.
                        groups = [(s, oi) for s in range(2) for oi in range(3)]
                        psr = {
                            (s, oi): pspool.tile(
                                [P, 512], f32, name=f"ps_r{s}_{oi}", tag="ps"
                            )
                            for (s, oi) in groups
                        }
                        for n in range(D_TILES):
                            for s, oi in groups:
                                ow = O_CHUNKS[oi][1]
                                nc.tensor.matmul(
                                    psr[(s, oi)][:, :ow],
                                    lhsT=xt[:, n, s * P : (s + 1) * P],
                                    rhs=wt[n][:, oi * 512 : oi * 512 + ow],
                                    start=(n == 0),
                                    stop=(n == D_TILES - 1),
                                )
                        for s in range(2):
                            copyback_and_store(
                                [psr[(s, oi)] for oi in range(3)], c, j, s
                            )
                        for s in (2, 3):
                            copyback_and_store(mm_subtile(s), c, j, s)
                    else:
                        for s in range(T_SUB):
                            copyback_and_store(mm_subtile(s), c, j, s)
    nc.finalize()
    _NC = nc
    return nc


def _host_prep(x, lora_id, W, Wd, Wu):
    x = np.asarray(x, dtype=np.float32)
    lora_id = np.asarray(lora_id)
    W = np.asarray(W, dtype=np.float32)
    Wd = np.asarray(Wd, dtype=np.float32)
    Wu = np.asarray(Wu, dtype=np.float32)

    idx = lora_id.astype(np.int64) // LORA_STRIDE
    active = lora_id >= 0
    safe_idx = np.where(active, idx, 0)

    WT = np.ascontiguousarray(W.T)  # [d, o]
    waugT = np.empty((G, D_IN, D_OUT), dtype=np.float16)
    for b in range(G):
        if active[b]:
            i = int(safe_idx[b])
            # (Wu[i] @ Wd[i]).T = Wd[i].T @ Wu[i].T : [d, o]
            waugT[b] = WT + SCALE * (Wd[i].T @ Wu[i].T)
        else:
            waugT[b] = WT

    # [G, d, t] — contraction dim first so SBUF tiles need no transpose
    xT = np.ascontiguousarray(x.transpose(0, 2, 1)).astype(np.float16)
    return xT, waugT


def kernel(x, lora_id, W, Wd, Wu):
    from concourse.bass_utils import run_bass_kernel_spmd

    xT, waugT = _host_prep(x, lora_id, W, Wd, Wu)

    nc = _build()
    in_maps = [
        {"xT": xT[k * CPC : (k + 1) * CPC], "wT": waugT[k * CPC : (k + 1) * CPC]}
        for k in range(N_CORES)
    ]
    trace = bool(os.environ.get("KERNEL_PROFILE"))
    kwargs = {}
    if trace and os.environ.get("KERNEL_PROFILE_DIR"):
        kwargs["tmpdir"] = os.environ["KERNEL_PROFILE_DIR"]
    res = run_bass_kernel_spmd(nc, in_maps, list(range(N_CORES)), trace=trace, **kwargs)
    if trace:
        kernel.last_results = res
        print(f"HW exec time: {res.exec_time_ns} ns")
    return np.concatenate(
        [res.results[k]["out"] for k in range(N_CORES)], axis=0
    ).astype(np.float32)
